# revision 27
# baseline (speedup 1.0000x reference)
"""Block-diagonal rotation (COB) kernel for Trainium2, 8 NeuronCores.

Computes out[..., block_i] = x[..., block_i] @ W_i.T for 8 square blocks of
sizes [512, 1024, 256, 768, 384, 640, 128, 384] (features sum to 4096),
x shape (4, 2048, 4096) fp32.

Strategy:
  - Pure data-parallel over rows: 8192 rows are split 8 ways (1024 rows/core).
    Each core gets all (host-pre-transposed) weights.
  - Weights are DMA'd once into SBUF and stay resident as float32r
    (TRN2's fast 4-byte matmul dtype: 1 cycle/row vs 4 for fp32,
    ~1.5e-4 max rel err at these contraction depths).
  - x tiles [128, 4096] are DMA'd naturally (rows on partitions), transposed
    128x128 on the TensorEngine (transpose mode), PSUM->SBUF copied by the
    VectorEngine, then used as the stationary operand of f32r matmuls
    against the resident weight tiles.  PSUM accumulates over each block's
    contraction dim; results are copied (alternating DVE/ACT) to an SBUF
    staging tile and DMA'd out in 1 MiB transfers per 128-row tile.
  - fp32 bits are fed directly into float32r tiles (verified bit-identical
    to explicitly rounded operands on HW - the PE rounds internally).
  - Software pipelining: transposes for row-tile r+2 are interleaved into
    row-tile r's block matmuls so the in-order TensorE stream stays busy
    while the weight preload streams in; weight chunks alternate between
    the two HWDGE rings (Scalar + Sync) to halve preload latency.

Measured on trn2 (8 cores): ~140-150 us HW exec (best 139.6 us; +-4%
run-to-run from HBM-neighbor noise), max rel err ~1.5e-4 (float32r's
~13-bit mantissa).  DMA-bound: 42.25 MiB/core at ~350 GB/s (DMA busy
~122 us, PE busy ~113 us, ~19 us residual PE stalls mostly from the
one-time weight preload and the fixed kernel drain).
"""

import numpy as np

import concourse.bacc as bacc
import concourse.mybir as mybir
from concourse.tile import TileContext
from concourse.bass_utils import run_bass_kernel_spmd
from concourse.masks import make_identity

SIZES = [512, 1024, 256, 768, 384, 640, 128, 384]
OFFS = np.cumsum([0] + SIZES)
N_CORES = 8
ROWS_TOTAL = 4 * 2048
ROWS_PER_CORE = ROWS_TOTAL // N_CORES  # 1024
D = 4096
P = 128
R_TILES = ROWS_PER_CORE // P  # 8

# e-slices per block: chunks <=512, all >=256 when possible (f32r matmul
# runs 1 cycle/row only for moving dim >= 256; 512 is the PSUM bank limit)
E_SLICES = {
    512: [512], 1024: [512, 512], 256: [256], 768: [512, 256],
    384: [384], 640: [384, 256], 128: [128],
}

F32R = mybir.dt.float32r
F32 = mybir.dt.float32

_cache = {}


def build_nc():
    if "nc" in _cache:
        return _cache["nc"]
    nc = bacc.Bacc()
    x_d = nc.declare_dram_parameter("x", [ROWS_PER_CORE, D], F32R, isOutput=False)
    w_d = [
        nc.declare_dram_parameter(f"w{i}", [s, s], F32R, isOutput=False)
        for i, s in enumerate(SIZES)
    ]
    out_d = nc.declare_dram_parameter("out", [ROWS_PER_CORE, D], F32, isOutput=True)

    x_v = x_d.rearrange("(r p) d -> r p d", p=P)
    out_v = out_d.rearrange("(r p) d -> r p d", p=P)

    with TileContext(nc, pool_alloc_mode="queue") as tc:
        with (
            tc.tile_pool(name="wres", bufs=1) as wres,
            tc.tile_pool(name="xnat", bufs=2) as xnat_p,
            tc.tile_pool(name="xt", bufs=3) as xt_p,
            tc.tile_pool(name="osb", bufs=2) as osb_p,
            tc.tile_pool(name="idp", bufs=1) as idp,
            tc.tile_pool(name="tp", bufs=2, space="PSUM") as tp_p,
            tc.tile_pool(name="mm", bufs=4, space="PSUM") as mm_p,
        ):
            # identity (f32r) for PE transpose
            id32 = idp.tile([P, P], F32, tag="id32")
            make_identity(nc, id32[:])
            ident = idp.tile([P, P], F32R, tag="idr")
            nc.vector.tensor_copy(ident[:], id32[:])

            # resident weights: per block, per k-tile: [128, s] f32r.
            # Even-numbered chunks stream on the Scalar-engine HWDGE ring
            # immediately; odd-numbered chunks go on the Sync ring, queued
            # right after the prologue x tiles (deferred emission below), so
            # the weight preload finishes roughly twice as fast while the
            # early x tiles still arrive first.
            wt = []
            w_sync_dmas = {i: [] for i in range(len(SIZES))}  # block -> [(tile, src)]
            ci = 0
            for i, s in enumerate(SIZES):
                w_v = w_d[i].rearrange("(k p) e -> k p e", p=P)
                ks = []
                for k in range(s // P):
                    t = wres.tile([P, s], F32R, tag=f"w{i}_{k}")
                    if ci % 2 == 0:
                        nc.scalar.dma_start(out=t[:], in_=w_v[k])
                    else:
                        w_sync_dmas[i].append((t, w_v[k]))
                    ks.append(t)
                    ci += 1
                wt.append(ks)

            # Software pipeline over row-tiles, demand-driven: before each
            # block's matmuls only the transpose groups it needs are
            # emitted; the lookahead pump runs AFTER the matmuls, keeping a
            # sliding window of up to 2 row-tiles of transposed x ahead of
            # the (in-order) TensorE matmul stream.  This lets the first
            # matmuls start as soon as x tile 0 and w0 arrive while the
            # weight preload is still streaming.
            xnat = {}  # r -> (lo_tile, hi_tile)
            xts_all = {}  # r -> {j: xt tile}

            def issue_x_dma(r, quarters=False):
                lo = xnat_p.tile([P, D // 2], F32R, tag="xnl")
                hi = xnat_p.tile([P, D // 2], F32R, tag="xnh")
                if quarters:
                    q = D // 4
                    nc.sync.dma_start(out=lo[:, :q], in_=x_v[r][:, :q])
                    nc.sync.dma_start(out=lo[:, q:], in_=x_v[r][:, q:2 * q])
                    nc.sync.dma_start(out=hi[:, :q], in_=x_v[r][:, 2 * q:3 * q])
                    nc.sync.dma_start(out=hi[:, q:], in_=x_v[r][:, 3 * q:])
                else:
                    nc.sync.dma_start(out=lo[:], in_=x_v[r][:, :D // 2])
                    nc.sync.dma_start(out=hi[:], in_=x_v[r][:, D // 2:])
                xnat[r] = (lo, hi)

            def transpose_group(r, j):
                # transposes d-tiles 4j..4j+3 of row-tile r into xt tile j
                lo, hi = xnat[r]
                src = lo if j < 4 else hi
                base = P * 4 * j - (0 if j < 4 else D // 2)
                ps = tp_p.tile([P, 4 * P], F32R, tag="tp")
                for i in range(4):
                    nc.tensor.transpose(
                        ps[:, P * i:P * (i + 1)],
                        src[:, base + P * i:base + P * (i + 1)],
                        ident[:],
                    )
                xt = xt_p.tile([P, 4 * P], F32R, tag=f"xt{j}")
                nc.vector.tensor_copy(xt[:], ps[:])
                xts_all.setdefault(r, {})[j] = xt

            # global ordered list of transpose groups and the pump cursor
            tp_queue = [(r, j) for r in range(R_TILES) for j in range(8)]
            state = {"cursor": 0}

            def pump_to(idx):
                # emit transpose groups up to global index idx (exclusive)
                idx = min(idx, len(tp_queue))
                while state["cursor"] < idx:
                    r_, j_ = tp_queue[state["cursor"]]
                    transpose_group(r_, j_)
                    state["cursor"] += 1

            # j-group needed to cover all d-tiles of block b
            J_HI = [(int(OFFS[b + 1]) - 1) // 512 for b in range(len(SIZES))]

            # Sync-ring queue order: x row-tile 0 first, then the sync-side
            # chunks of the first two blocks (needed by the very first
            # matmuls), then x row-tile 1, then the rest of the weights —
            # matching the order the in-order TensorE stream consumes them.
            issue_x_dma(0, quarters=True)
            for i in (0, 1):
                for t, src in w_sync_dmas[i]:
                    nc.sync.dma_start(out=t[:], in_=src)
            issue_x_dma(1)
            for i in range(2, len(SIZES)):
                for t, src in w_sync_dmas[i]:
                    nc.sync.dma_start(out=t[:], in_=src)

            for r in range(R_TILES):
                if r + 2 < R_TILES:
                    issue_x_dma(r + 2)
                o_t = osb_p.tile([P, D], F32, tag="os")
                for b, s in enumerate(SIZES):
                    # required groups for this block, then the matmuls,
                    # then pump the lookahead window (<= 2 row-tiles ahead,
                    # matching the xt pool's bufs=3)
                    pump_to(r * 8 + J_HI[b] + 1)
                    xts = xts_all[r]
                    d0 = int(OFFS[b]) // P  # first global d-tile of block
                    kt = s // P
                    n0 = 0
                    for nw in E_SLICES[s]:
                        ps = mm_p.tile([P, nw], F32, tag="mm", name="mmps")
                        for k in range(kt):
                            g = d0 + k
                            lhsT = xts[g // 4][:, P * (g % 4):P * (g % 4 + 1)]
                            nc.tensor.matmul(
                                ps[:], lhsT, wt[b][k][:, n0:n0 + nw],
                                start=(k == 0), stop=(k == kt - 1),
                            )
                        # alternate PSUM->SBUF output copies between DVE and
                        # ACT so neither engine becomes the bottleneck
                        dst = o_t[:, int(OFFS[b]) + n0:int(OFFS[b]) + n0 + nw]
                        if (r + b) % 2 == 0:
                            nc.scalar.copy(dst, ps[:])
                        else:
                            nc.vector.tensor_copy(dst, ps[:])
                        n0 += nw
                    pump_to(r * 8 + b + 17)
                del xts_all[r]
                if r == R_TILES - 1:
                    # finer final-out chunks so the tail DMA drains sooner
                    q = D // 4
                    for c in range(4):
                        nc.sync.dma_start(out=out_v[r][:, c * q:(c + 1) * q],
                                          in_=o_t[:, c * q:(c + 1) * q])
                else:
                    nc.sync.dma_start(out=out_v[r][:, :D // 2], in_=o_t[:, :D // 2])
                    nc.sync.dma_start(out=out_v[r][:, D // 2:], in_=o_t[:, D // 2:])

    nc.finalize()
    _cache["nc"] = nc
    return nc


def build_in_maps(x, w0, w1, w2, w3, w4, w5, w6, w7):
    x = np.ascontiguousarray(np.asarray(x, dtype=np.float32)).reshape(ROWS_TOTAL, D)
    ws = [w0, w1, w2, w3, w4, w5, w6, w7]
    wts = [
        np.ascontiguousarray(np.asarray(w, dtype=np.float32).T) for w in ws
    ]
    in_maps = []
    for c in range(N_CORES):
        m = {"x": x[c * ROWS_PER_CORE:(c + 1) * ROWS_PER_CORE]}
        for i, wt in enumerate(wts):
            m[f"w{i}"] = wt
        in_maps.append(m)
    return in_maps


def kernel(x, w0, w1, w2, w3, w4, w5, w6, w7):
    nc = build_nc()
    in_maps = build_in_maps(x, w0, w1, w2, w3, w4, w5, w6, w7)
    res = run_bass_kernel_spmd(nc, in_maps, list(range(N_CORES)))
    out = np.concatenate([r["out"] for r in res.results], axis=0)
    return out.reshape(4, 2048, D).astype(np.float32, copy=False)


# revision 28
# speedup vs baseline: 1.0408x; 1.0408x over previous
"""Block-diagonal rotation (COB) kernel for Trainium2, 8 NeuronCores.

Computes out[..., block_i] = x[..., block_i] @ W_i.T for 8 square blocks of
sizes [512, 1024, 256, 768, 384, 640, 128, 384] (features sum to 4096),
x shape (4, 2048, 4096) fp32.

Strategy:
  - Pure data-parallel over rows: 8192 rows are split 8 ways (1024 rows/core).
    Each core gets all (host-pre-transposed) weights.
  - Weights are DMA'd once into SBUF and stay resident as float32r
    (TRN2's fast 4-byte matmul dtype: 1 cycle/row vs 4 for fp32,
    ~1.5e-4 max rel err at these contraction depths).
  - x tiles [128, 4096] are DMA'd naturally (rows on partitions), transposed
    128x128 on the TensorEngine (transpose mode), PSUM->SBUF copied by the
    VectorEngine, then used as the stationary operand of f32r matmuls
    against the resident weight tiles.  PSUM accumulates over each block's
    contraction dim; results are copied (alternating DVE/ACT) to an SBUF
    staging tile and DMA'd out in 1 MiB transfers per 128-row tile.
  - fp32 bits are fed directly into float32r tiles (verified bit-identical
    to explicitly rounded operands on HW - the PE rounds internally).
  - Software pipelining: transposes for row-tile r+2 are interleaved into
    row-tile r's block matmuls so the in-order TensorE stream stays busy
    while the weight preload streams in; weight chunks alternate between
    the two HWDGE rings (Scalar + Sync) to halve preload latency.

Measured on trn2 (8 cores): ~140-150 us HW exec (best 139.6 us; +-4%
run-to-run from HBM-neighbor noise), max rel err ~1.5e-4 (float32r's
~13-bit mantissa).  DMA-bound: 42.25 MiB/core at ~350 GB/s (DMA busy
~122 us, PE busy ~113 us, ~19 us residual PE stalls mostly from the
one-time weight preload and the fixed kernel drain).
"""

import numpy as np

import concourse.bacc as bacc
import concourse.mybir as mybir
from concourse.tile import TileContext
from concourse.bass_utils import run_bass_kernel_spmd
from concourse.masks import make_identity

SIZES = [512, 1024, 256, 768, 384, 640, 128, 384]
OFFS = np.cumsum([0] + SIZES)
N_CORES = 8
ROWS_TOTAL = 4 * 2048
ROWS_PER_CORE = ROWS_TOTAL // N_CORES  # 1024
D = 4096
P = 128
R_TILES = ROWS_PER_CORE // P  # 8

# e-slices per block: chunks <=512, all >=256 when possible (f32r matmul
# runs 1 cycle/row only for moving dim >= 256; 512 is the PSUM bank limit)
E_SLICES = {
    512: [512], 1024: [512, 512], 256: [256], 768: [512, 256],
    384: [384], 640: [384, 256], 128: [128],
}

F32R = mybir.dt.float32r
F32 = mybir.dt.float32

_cache = {}


def build_nc():
    if "nc" in _cache:
        return _cache["nc"]
    nc = bacc.Bacc()
    x_d = nc.declare_dram_parameter("x", [ROWS_PER_CORE, D], F32R, isOutput=False)
    w_d = [
        nc.declare_dram_parameter(f"w{i}", [s, s], F32R, isOutput=False)
        for i, s in enumerate(SIZES)
    ]
    out_d = nc.declare_dram_parameter("out", [ROWS_PER_CORE, D], F32, isOutput=True)

    x_v = x_d.rearrange("(r p) d -> r p d", p=P)
    out_v = out_d.rearrange("(r p) d -> r p d", p=P)

    with TileContext(nc) as tc:
        with (
            tc.tile_pool(name="wres", bufs=1) as wres,
            tc.tile_pool(name="xnat", bufs=2) as xnat_p,
            tc.tile_pool(name="xt", bufs=3) as xt_p,
            tc.tile_pool(name="osb", bufs=2) as osb_p,
            tc.tile_pool(name="idp", bufs=1) as idp,
            tc.tile_pool(name="tp", bufs=2, space="PSUM") as tp_p,
            tc.tile_pool(name="mm", bufs=4, space="PSUM") as mm_p,
        ):
            # identity (f32r) for PE transpose
            id32 = idp.tile([P, P], F32, tag="id32")
            make_identity(nc, id32[:])
            ident = idp.tile([P, P], F32R, tag="idr")
            nc.vector.tensor_copy(ident[:], id32[:])

            # resident weights: per block, per k-tile: [128, s] f32r.
            # Even-numbered chunks stream on the Scalar-engine HWDGE ring
            # immediately; odd-numbered chunks go on the Sync ring, queued
            # right after the prologue x tiles (deferred emission below), so
            # the weight preload finishes roughly twice as fast while the
            # early x tiles still arrive first.
            wt = []
            w_sync_dmas = {i: [] for i in range(len(SIZES))}  # block -> [(tile, src)]
            ci = 0
            for i, s in enumerate(SIZES):
                w_v = w_d[i].rearrange("(k p) e -> k p e", p=P)
                ks = []
                for k in range(s // P):
                    t = wres.tile([P, s], F32R, tag=f"w{i}_{k}")
                    if ci % 2 == 0:
                        nc.scalar.dma_start(out=t[:], in_=w_v[k])
                    else:
                        w_sync_dmas[i].append((t, w_v[k]))
                    ks.append(t)
                    ci += 1
                wt.append(ks)

            # Software pipeline over row-tiles, demand-driven: before each
            # block's matmuls only the transpose groups it needs are
            # emitted; the lookahead pump runs AFTER the matmuls, keeping a
            # sliding window of up to 2 row-tiles of transposed x ahead of
            # the (in-order) TensorE matmul stream.  This lets the first
            # matmuls start as soon as x tile 0 and w0 arrive while the
            # weight preload is still streaming.
            xnat = {}  # r -> (lo_tile, hi_tile)
            xts_all = {}  # r -> {j: xt tile}

            def issue_x_dma(r, quarters=False):
                lo = xnat_p.tile([P, D // 2], F32R, tag="xnl")
                hi = xnat_p.tile([P, D // 2], F32R, tag="xnh")
                if quarters:
                    q = D // 4
                    nc.sync.dma_start(out=lo[:, :q], in_=x_v[r][:, :q])
                    nc.sync.dma_start(out=lo[:, q:], in_=x_v[r][:, q:2 * q])
                    nc.sync.dma_start(out=hi[:, :q], in_=x_v[r][:, 2 * q:3 * q])
                    nc.sync.dma_start(out=hi[:, q:], in_=x_v[r][:, 3 * q:])
                else:
                    nc.sync.dma_start(out=lo[:], in_=x_v[r][:, :D // 2])
                    nc.sync.dma_start(out=hi[:], in_=x_v[r][:, D // 2:])
                xnat[r] = (lo, hi)

            def transpose_group(r, j):
                # transposes d-tiles 4j..4j+3 of row-tile r into xt tile j
                lo, hi = xnat[r]
                src = lo if j < 4 else hi
                base = P * 4 * j - (0 if j < 4 else D // 2)
                ps = tp_p.tile([P, 4 * P], F32R, tag="tp")
                for i in range(4):
                    nc.tensor.transpose(
                        ps[:, P * i:P * (i + 1)],
                        src[:, base + P * i:base + P * (i + 1)],
                        ident[:],
                    )
                xt = xt_p.tile([P, 4 * P], F32R, tag=f"xt{j}")
                nc.vector.tensor_copy(xt[:], ps[:])
                xts_all.setdefault(r, {})[j] = xt

            # global ordered list of transpose groups and the pump cursor
            tp_queue = [(r, j) for r in range(R_TILES) for j in range(8)]
            state = {"cursor": 0}

            def pump_to(idx):
                # emit transpose groups up to global index idx (exclusive)
                idx = min(idx, len(tp_queue))
                while state["cursor"] < idx:
                    r_, j_ = tp_queue[state["cursor"]]
                    transpose_group(r_, j_)
                    state["cursor"] += 1

            # j-group needed to cover all d-tiles of block b
            J_HI = [(int(OFFS[b + 1]) - 1) // 512 for b in range(len(SIZES))]

            # Sync-ring queue order: x row-tile 0 first, then the sync-side
            # chunks of the first two blocks (needed by the very first
            # matmuls), then x row-tile 1, then the rest of the weights —
            # matching the order the in-order TensorE stream consumes them.
            issue_x_dma(0, quarters=True)
            for i in (0, 1):
                for t, src in w_sync_dmas[i]:
                    nc.sync.dma_start(out=t[:], in_=src)
            issue_x_dma(1)
            for i in range(2, len(SIZES)):
                for t, src in w_sync_dmas[i]:
                    nc.sync.dma_start(out=t[:], in_=src)

            for r in range(R_TILES):
                if r + 2 < R_TILES:
                    issue_x_dma(r + 2)
                o_t = osb_p.tile([P, D], F32, tag="os")
                for b, s in enumerate(SIZES):
                    # required groups for this block, then the matmuls,
                    # then pump the lookahead window (<= 2 row-tiles ahead,
                    # matching the xt pool's bufs=3)
                    pump_to(r * 8 + J_HI[b] + 1)
                    xts = xts_all[r]
                    d0 = int(OFFS[b]) // P  # first global d-tile of block
                    kt = s // P
                    n0 = 0
                    for nw in E_SLICES[s]:
                        ps = mm_p.tile([P, nw], F32, tag="mm", name="mmps")
                        for k in range(kt):
                            g = d0 + k
                            lhsT = xts[g // 4][:, P * (g % 4):P * (g % 4 + 1)]
                            nc.tensor.matmul(
                                ps[:], lhsT, wt[b][k][:, n0:n0 + nw],
                                start=(k == 0), stop=(k == kt - 1),
                            )
                        # alternate PSUM->SBUF output copies between DVE and
                        # ACT so neither engine becomes the bottleneck
                        dst = o_t[:, int(OFFS[b]) + n0:int(OFFS[b]) + n0 + nw]
                        if (r + b) % 2 == 0:
                            nc.scalar.copy(dst, ps[:])
                        else:
                            nc.vector.tensor_copy(dst, ps[:])
                        n0 += nw
                    pump_to(r * 8 + b + 17)
                del xts_all[r]
                if r == R_TILES - 1:
                    # finer final-out chunks so the tail DMA drains sooner
                    q = D // 4
                    for c in range(4):
                        nc.sync.dma_start(out=out_v[r][:, c * q:(c + 1) * q],
                                          in_=o_t[:, c * q:(c + 1) * q])
                else:
                    nc.sync.dma_start(out=out_v[r][:, :D // 2], in_=o_t[:, :D // 2])
                    nc.sync.dma_start(out=out_v[r][:, D // 2:], in_=o_t[:, D // 2:])

    nc.finalize()
    _cache["nc"] = nc
    return nc


def build_in_maps(x, w0, w1, w2, w3, w4, w5, w6, w7):
    x = np.ascontiguousarray(np.asarray(x, dtype=np.float32)).reshape(ROWS_TOTAL, D)
    ws = [w0, w1, w2, w3, w4, w5, w6, w7]
    wts = [
        np.ascontiguousarray(np.asarray(w, dtype=np.float32).T) for w in ws
    ]
    in_maps = []
    for c in range(N_CORES):
        m = {"x": x[c * ROWS_PER_CORE:(c + 1) * ROWS_PER_CORE]}
        for i, wt in enumerate(wts):
            m[f"w{i}"] = wt
        in_maps.append(m)
    return in_maps


def kernel(x, w0, w1, w2, w3, w4, w5, w6, w7):
    nc = build_nc()
    in_maps = build_in_maps(x, w0, w1, w2, w3, w4, w5, w6, w7)
    res = run_bass_kernel_spmd(nc, in_maps, list(range(N_CORES)))
    out = np.concatenate([r["out"] for r in res.results], axis=0)
    return out.reshape(4, 2048, D).astype(np.float32, copy=False)
